# revision 42
# baseline (speedup 1.0000x reference)
"""CSPNet GNN message-passing kernel for trn2 (8 NeuronCores, SPMD data-parallel).

Structure exploited (from the reference generator):
  - B=64 crystals x A=32 atoms, fully-connected edges per crystal in row-major
    (src-major) order => gathers become free-axis broadcasts, scatter-mean
    becomes a grouped reduction over 32 consecutive edges.
  - edge-MLP layer 1 decomposes: ein @ W1 = Hi[src] + Hj[dst] + (fd,lat,1) @ Wc
    where Hi/Hj are small per-node projections broadcast across edges.

Layout: activations stored feature-on-partitions [512f -> 4x128, nodes/edges on
free axis]. Matmuls in bf16 (fp32 PSUM accumulate), masters in fp32.
"""
import os
import numpy as np
import ml_dtypes

# ---- problem constants (hardcoded per contract) ----
B, A = 64, 32
N, E = B * A, B * A * A
H, LAT = 512, 256
L = 4
NF = 10
MAXA = 100
DIS = 2 * NF * 3          # 60
EIN = 2 * H + 9 + DIS     # 1093
NCORES = 8
GPC = B // NCORES         # graphs per core = 8
NPC = GPC * A             # nodes per core = 256
EPC = GPC * A * A         # edges per core = 8192
NCHUNK = 16               # edge chunks per core (512 edges = half a graph)
CH = EPC // NCHUNK        # 512
MT = H // 128             # feature tiles = 4

_bf16 = ml_dtypes.bfloat16


# ---------------------------------------------------------------------------
# weight/const blob layout (shared by host packer and device program)
# ---------------------------------------------------------------------------
class _Layout:
    def __init__(self):
        self.cols = 0
        self.off = {}
        # W_latent [768,512] -> 6 K-tiles
        self._add("wlat", 6 * H)
        self._add("emb", H)                     # K=100
        for i in range(L):
            self._add(f"w1hi{i}", 4 * H)
            self._add(f"w1hj{i}", 4 * H)
            self._add(f"wc{i}", H)              # K=60 (fd rows)
            self._add(f"wcl{i}", H)             # K=10 ([b1; lat9])
            self._add(f"w2{i}", 4 * H)
            self._add(f"nw1{i}", 8 * H)
            self._add(f"nw2{i}", 4 * H)
        self._add("coordw", 4 * 3)
        self._add("latw", 4 * 9)
        self._add("ident", 128)

    def _add(self, name, n):
        self.off[name] = self.cols
        self.cols += n


class _CLayout:
    def __init__(self):
        self.cols = 0
        self.off = {}
        self._add("blat", MT)
        for i in range(L):
            self._add(f"b2_{i}", MT)
            self._add(f"nb1_{i}", MT)
            self._add(f"nb2_{i}", MT)
        self._add("sel", 124)  # [67, 124]: rows 0-2 / 64-66 -> cols 0-59 / 64-123
        self._add("shift", 1)   # rows 0-59 / 64-123: +0.25 for cos rows
        self._add("id8", 8)
        self._add("id9", 9)

    def _add(self, name, n):
        self.off[name] = self.cols
        self.cols += n


_WL = _Layout()
_CL = _CLayout()


def _pack_kxm(dst, col0, w):
    """Pack w [K, M] as lhsT tiles into dst[128, :] at col0: K-tile kt spans
    cols [col0 + kt*M, col0 + (kt+1)*M), rows 0..min(128, K - kt*128)."""
    K, M = w.shape
    nkt = (K + 127) // 128
    for kt in range(nkt):
        k0, k1 = kt * 128, min((kt + 1) * 128, K)
        dst[0:k1 - k0, col0 + kt * M: col0 + kt * M + M] = w[k0:k1, :]


def _host_blobs(inputs):
    """Build the weight blob (bf16) and const blob (f32), shared by all cores."""
    wb = np.zeros((128, _WL.cols), np.float32)
    o = _WL.off
    _pack_kxm(wb, o["wlat"], np.asarray(inputs["W_latent"]))
    _pack_kxm(wb, o["emb"], np.asarray(inputs["emb_table"]))
    for i in range(L):
        W1 = np.asarray(inputs["edge_W1"][i])    # [1093, 512]
        _pack_kxm(wb, o[f"w1hi{i}"], W1[0:H])
        _pack_kxm(wb, o[f"w1hj{i}"], W1[H:2 * H])
        # wc: fd rows only (reference order); wcl: [b1; lat rows]
        _pack_kxm(wb, o[f"wc{i}"], W1[2 * H + 9:])
        wcl = np.zeros((10, H), np.float32)
        wcl[0] = np.asarray(inputs["edge_b1"][i])
        wcl[1:10] = W1[2 * H:2 * H + 9]
        _pack_kxm(wb, o[f"wcl{i}"], wcl)
        _pack_kxm(wb, o[f"w2{i}"], np.asarray(inputs["edge_W2"][i]))
        nw1 = np.asarray(inputs["node_W1"][i]).copy()
        nw1[H:] /= A                              # fold scatter-mean /32
        _pack_kxm(wb, o[f"nw1{i}"], nw1)
        _pack_kxm(wb, o[f"nw2{i}"], np.asarray(inputs["node_W2"][i]))
    _pack_kxm(wb, o["coordw"], np.asarray(inputs["coord_W"]))
    _pack_kxm(wb, o["latw"], np.asarray(inputs["lattice_W"]) / A)  # fold graph mean

    cb = np.zeros((128, _CL.cols), np.float32)
    co = _CL.off
    cb[:, co["blat"]:co["blat"] + MT] = np.asarray(inputs["b_latent"]).reshape(MT, 128).T
    for i in range(L):
        cb[:, co[f"b2_{i}"]:co[f"b2_{i}"] + MT] = np.asarray(inputs["edge_b2"][i]).reshape(MT, 128).T
        cb[:, co[f"nb1_{i}"]:co[f"nb1_{i}"] + MT] = np.asarray(inputs["node_b1"][i]).reshape(MT, 128).T
        cb[:, co[f"nb2_{i}"]:co[f"nb2_{i}"] + MT] = np.asarray(inputs["node_b2"][i]).reshape(MT, 128).T
    # selection matrix for fd, packed 2 graphs per matmul:
    # diff rows 0-2 (graph A) -> kd rows 0-59; rows 64-66 (graph B) -> 64-123
    sel = np.zeros((67, 124), np.float32)
    shift = np.zeros((128,), np.float32)
    for r in range(DIS):
        is_cos = r >= 30
        rr = r - 30 if is_cos else r
        c, k = rr // NF, rr % NF
        sel[c, r] = float(k)
        sel[64 + c, 64 + r] = float(k)
        if is_cos:
            shift[r] = 0.25
            shift[64 + r] = 0.25
    cb[0:67, co["sel"]:co["sel"] + 124] = sel
    cb[:, co["shift"]] = shift
    wb[:, o["ident"]:o["ident"] + 128] = np.eye(128, dtype=np.float32)
    cb[0:8, co["id8"]:co["id8"] + 8] = np.eye(8, dtype=np.float32)
    cb[0:9, co["id9"]:co["id9"] + 9] = np.eye(9, dtype=np.float32)
    return wb.astype(_bf16), cb


# ---------------------------------------------------------------------------
# structure validation + fallback
# ---------------------------------------------------------------------------
def _structure_ok(inputs):
    ei = np.asarray(inputs["edge_index"])
    n2g = np.asarray(inputs["node2graph"])
    if ei.shape != (2, E) or n2g.shape != (N,):
        return False
    if not np.array_equal(n2g, np.repeat(np.arange(B, dtype=n2g.dtype), A)):
        return False
    base = (np.arange(B, dtype=np.int64) * A)[:, None]
    ii = np.repeat(np.arange(A), A)
    jj = np.tile(np.arange(A), A)
    src = (base + ii[None, :]).reshape(-1)
    dst = (base + jj[None, :]).reshape(-1)
    return np.array_equal(ei[0], src) and np.array_equal(ei[1], dst)


def _silu(x):
    return x / (1.0 + np.exp(-x))


def _fallback(inputs):
    """Pure numpy reference for arbitrary edge_index (correctness insurance)."""
    t = np.asarray(inputs["t"]); frac = np.asarray(inputs["frac_coords"])
    latt = np.asarray(inputs["lattices"])
    ei = np.asarray(inputs["edge_index"]); n2g = np.asarray(inputs["node2graph"])
    src, dst = ei[0], ei[1]
    fd = (frac[dst] - frac[src]) % 1.0
    freqs = 2.0 * np.pi * np.arange(NF, dtype=np.float32)
    fdv = (fd[:, :, None] * freqs).reshape(-1, 3 * NF)
    fde = np.concatenate([np.sin(fdv), np.cos(fdv)], -1)
    lat_ip = np.einsum('bij,bkj->bik', latt, latt).reshape(-1, 9)
    lat_e = lat_ip[n2g[src]]
    nf = np.asarray(inputs["emb_table"])[np.asarray(inputs["atom_types"]) - 1]
    nf = np.concatenate([nf, t[n2g]], 1) @ np.asarray(inputs["W_latent"]) + np.asarray(inputs["b_latent"])
    ones_e = np.ones((src.shape[0], 1), np.float32)
    for i in range(L):
        ein = np.concatenate([nf[src], nf[dst], lat_e, fde], 1)
        ef = _silu(_silu(ein @ np.asarray(inputs["edge_W1"][i]) + np.asarray(inputs["edge_b1"][i]))
                   @ np.asarray(inputs["edge_W2"][i]) + np.asarray(inputs["edge_b2"][i]))
        agg = np.zeros((N, H), np.float32); np.add.at(agg, src, ef)
        deg = np.zeros((N, 1), np.float32); np.add.at(deg, src, ones_e)
        agg = agg / np.maximum(deg, 1.0)
        nin = np.concatenate([nf, agg], 1)
        nf = nf + _silu(_silu(nin @ np.asarray(inputs["node_W1"][i]) + np.asarray(inputs["node_b1"][i]))
                        @ np.asarray(inputs["node_W2"][i]) + np.asarray(inputs["node_b2"][i]))
    coord_out = nf @ np.asarray(inputs["coord_W"])
    gf = nf.reshape(B, A, H).mean(1)
    lat_out = (gf @ np.asarray(inputs["lattice_W"])).reshape(-1, 3, 3)
    lat_out = np.einsum('bij,bjk->bik', lat_out, latt)
    return (lat_out.astype(np.float32), coord_out.astype(np.float32))


# ---------------------------------------------------------------------------
# wait legalization (walrus allows at most 1 semaphore wait per instruction)
# ---------------------------------------------------------------------------
def _legalize_waits(nc, max_waits=1):
    import concourse.mybir as mybir
    n_fixed = 0
    for fn in nc.m.functions:
        for blk in fn.blocks:
            insts = blk.instructions
            new_list = []
            changed = False
            for inst in insts:
                si = inst.sync_info
                waits = list(si.on_wait) if si is not None else []
                if len(waits) > max_waits:
                    keep = waits[-max_waits:]
                    for w in waits[:-max_waits]:
                        ev = mybir.InstEventSemaphore(
                            name=f"{inst.name}_hw{n_fixed}", ins=[], outs=[])
                        ev.engine = inst.engine
                        ev.sync_info = mybir.SyncInfo(on_wait=[w], on_update=[])
                        new_list.append(ev)
                        n_fixed += 1
                    si.on_wait = keep
                    inst.sync_info = si
                    changed = True
                new_list.append(inst)
            if changed:
                blk.instructions = new_list
    return n_fixed


# ---------------------------------------------------------------------------
# device program
# ---------------------------------------------------------------------------
CQ = 256                   # edges per chunk (quarter graph)
NCH = EPC // CQ            # 32 chunks per core per layer


DEBUG = False


def _build_program(zero_bias=False):
    from contextlib import ExitStack
    import concourse.bass as bass
    import concourse.mybir as mybir
    import concourse.tile as tile

    dt = mybir.dt
    AF = mybir.ActivationFunctionType
    AX = mybir.AxisListType
    OP = mybir.AluOpType
    o, co = _WL.off, _CL.off

    nc = bass.Bass()
    wb_d = nc.dram_tensor("wb", [128, _WL.cols], dt.bfloat16, kind="ExternalInput")
    cb_d = nc.dram_tensor("cb", [128, _CL.cols], dt.float32, kind="ExternalInput")
    fr_d = nc.dram_tensor("fr", [3, NPC], dt.float32, kind="ExternalInput")
    oh_d = nc.dram_tensor("oh", [100, NPC], dt.bfloat16, kind="ExternalInput")
    tt_d = nc.dram_tensor("tt", [LAT, GPC], dt.bfloat16, kind="ExternalInput")
    lt_d = nc.dram_tensor("lt", [GPC, 9], dt.float32, kind="ExternalInput")
    co_d = nc.dram_tensor("co", [3, NPC], dt.float32, kind="ExternalOutput")
    lo_d = nc.dram_tensor("lo", [GPC, 9], dt.float32, kind="ExternalOutput")
    if DEBUG:
        dbg_fd = nc.dram_tensor("dbg_fd", [DIS, EPC], dt.bfloat16, kind="ExternalOutput")
        dbg_hi = nc.dram_tensor("dbg_hi", [128, MT, NPC], dt.bfloat16, kind="ExternalOutput")
        dbg_hj = nc.dram_tensor("dbg_hj", [128, MT, NPC], dt.bfloat16, kind="ExternalOutput")
        dbg_z1s = nc.dram_tensor("dbg_z1s", [128, MT, CQ], dt.bfloat16, kind="ExternalOutput")
        dbg_agg = nc.dram_tensor("dbg_agg", [128, MT, NPC], dt.bfloat16, kind="ExternalOutput")
        dbg_nf = nc.dram_tensor("dbg_nf", [128, MT, NPC], dt.float32, kind="ExternalOutput")
        dbg_nf0 = nc.dram_tensor("dbg_nf0", [128, MT, NPC], dt.float32, kind="ExternalOutput")

    with ExitStack() as ctx:
        tc = ctx.enter_context(tile.TileContext(nc))
        pers = ctx.enter_context(tc.tile_pool(name="pers", bufs=1))
        work = ctx.enter_context(tc.tile_pool(name="work", bufs=2))
        ps = ctx.enter_context(tc.tile_pool(name="ps", bufs=2, space="PSUM"))

        # ---- persistent loads; weight blob split so layer-k compute can
        # start before later layers' weights arrive.
        cb = pers.tile([128, _CL.cols], dt.float32)
        nc.sync.dma_start(cb[:], cb_d[:, :])
        fr = pers.tile([3, NPC], dt.float32)
        nc.sync.dma_start(fr[:], fr_d[:, :])
        oh = pers.tile([100, NPC], dt.bfloat16)
        nc.sync.dma_start(oh[:], oh_d[:, :])
        tt = pers.tile([128, 2, GPC], dt.bfloat16)
        nc.sync.dma_start(tt[:], tt_d[:, :].rearrange("(kt p) c -> p kt c", p=128))
        lt = pers.tile([GPC, 9], dt.float32)
        nc.sync.dma_start(lt[:], lt_d[:, :])
        wb = pers.tile([128, _WL.cols], dt.bfloat16)
        layer_cuts = [o[f"w1hi{i}"] for i in range(L)] + [o["coordw"]]
        spans = [(0, o["w1hi0"]), (o["coordw"], _WL.cols)] + \
            list(zip(layer_cuts[:-1], layer_cuts[1:]))
        for a, b in spans:
            nc.sync.dma_start(wb[:, a:b], wb_d[:, a:b])

        rhs_c = pers.tile([DIS, EPC], dt.bfloat16)  # fd rows x edges
        nf_f = pers.tile([128, MT, NPC], dt.float32)
        nf_b = pers.tile([128, MT, NPC], dt.bfloat16)

        def wB(name, kt, m, M=H):
            c0 = o[name] + kt * M + m * 128
            return wb[:, c0:c0 + 128]

        def cB(name, m):
            c0 = co[name] + m
            return cb[:, c0:c0 + 1]

        ident = wb[:, o["ident"]:o["ident"] + 128]

        # ---- lat_ip = einsum('bij,bkj->bik') into cols 1..9 of [8, 10];
        # col 0 = ones (pairs with the b1 row of wcl). Transposed via PE to
        # latip1 [10, 8], which feeds the per-layer (lat,b1) bias vectors.
        lat_aug = work.tile([GPC, 10], dt.float32, tag="lat10", bufs=1)
        lat_t0 = work.tile([GPC, 9], dt.float32, tag="lat9b", bufs=1)
        lt3d = lt[:].rearrange("g (i j) -> g i j", i=3)

        def lat_view_i(j):
            return lt3d[:, :, j].unsqueeze(2).broadcast_to([GPC, 3, 3])

        def lat_view_k(j):
            return lt3d[:, :, j].unsqueeze(1).broadcast_to([GPC, 3, 3])

        nc.scalar.activation(lat_aug[:, 0:1], cb[0:GPC, 0:1], AF.Copy,
                             bias=1.0, scale=0.0)
        a3 = lat_aug[:, 1:10].rearrange("g (i k) -> g i k", i=3)
        t3 = lat_t0[:].rearrange("g (i k) -> g i k", i=3)
        nc.vector.tensor_mul(a3, lat_view_i(0), lat_view_k(0))
        nc.vector.tensor_mul(t3, lat_view_i(1), lat_view_k(1))
        nc.vector.tensor_add(a3, lat_aug[:, 1:10].rearrange("g (i k) -> g i k", i=3), t3)
        nc.vector.tensor_mul(t3, lat_view_i(2), lat_view_k(2))
        nc.vector.tensor_add(a3, lat_aug[:, 1:10].rearrange("g (i k) -> g i k", i=3), t3)
        latip1_ps = ps.tile([10, GPC], dt.float32, tag="z1")
        nc.tensor.matmul(latip1_ps[:], lat_aug[:], cb[0:8, co["id8"]:co["id8"] + 8],
                         start=True, stop=True)
        latip1 = work.tile([10, GPC], dt.bfloat16, tag="latip1", bufs=1)
        nc.vector.tensor_copy(latip1[:], latip1_ps[:])

        # ---- nf0 = [emb_gather; t_bcast] @ W_latent + b_latent
        x1_ps = ps.tile([128, MT, NPC], dt.float32, tag="z1")
        for m in range(MT):
            nc.tensor.matmul(x1_ps[:, m, :], wb[0:100, o["emb"] + m * 128: o["emb"] + m * 128 + 128],
                             oh[:], start=True, stop=True)
        x1 = work.tile([128, MT, NPC], dt.bfloat16, tag="x1", bufs=1)
        nc.scalar.copy(x1[:], x1_ps[:])
        x2 = work.tile([128, 2, NPC], dt.bfloat16, tag="x2", bufs=1)
        for kt in range(2):
            nc.vector.tensor_copy(
                x2[:, kt, :].rearrange("p (g j) -> p g j", j=A),
                tt[:, kt, :, None].broadcast_to([128, GPC, A]))
        nf_ps = ps.tile([128, MT, NPC], dt.float32, tag="ef")
        for m in range(MT):
            for kt in range(6):
                rhs = x1[:, kt, :] if kt < 4 else x2[:, kt - 4, :]
                nc.tensor.matmul(nf_ps[:, m, :], wB("wlat", kt, m), rhs,
                                 start=(kt == 0), stop=(kt == 5))
        if zero_bias:
            nc.scalar.activation(nf_f[:], nf_ps[:], AF.Identity)
        else:
            for m in range(MT):
                nc.scalar.activation(nf_f[:, m, :], nf_ps[:, m, :], AF.Identity,
                                     bias=cB("blat", m))
        nc.vector.tensor_copy(nf_b[:], nf_f[:])

        # ---- fd rows: sin/cos(2*pi*k*diff) via int-round range reduction.
        # Two graphs packed per pass: graph 2t -> partitions 0-2 (kd 0-59),
        # graph 2t+1 -> partitions 64-66 (kd 64-123).
        GG = A * A  # 1024 edges per graph
        diff2 = pers.tile([67, GG], dt.float32)
        nc.vector.memset(diff2[0:64, :], 0.0)
        nc.vector.memset(diff2[64:67, :], 0.0)
        def emit_fd(t):
            for h, gg in ((0, 2 * t), (64, 2 * t + 1)):
                fr_j = fr[:, gg * A:(gg + 1) * A][:, None, :].broadcast_to([3, A, A])
                fr_i = fr[:, gg * A:(gg + 1) * A][:, :, None].broadcast_to([3, A, A])
                nc.vector.tensor_sub(
                    diff2[h:h + 3, :].rearrange("p (i j) -> p i j", j=A), fr_j, fr_i)
            for half in range(2):
                kd_ps = ps.tile([124, CH], dt.float32, tag="z1", name=f"kd_{t}_{half}")
                nc.tensor.matmul(kd_ps[:], cb[0:67, co["sel"]:co["sel"] + 124],
                                 diff2[:, half * CH:(half + 1) * CH],
                                 start=True, stop=True)
                kd2 = work.tile([124, CH], dt.float32, tag="kd2")
                nc.vector.tensor_scalar_add(kd2[:], kd_ps[:],
                                            cb[0:124, co["shift"]:co["shift"] + 1])
                kint = work.tile([124, CH], dt.int32, tag="kint")
                nc.vector.tensor_copy(kint[:], kd2[:])
                kf = work.tile([124, CH], dt.float32, tag="kf")
                nc.vector.tensor_copy(kf[:], kint[:])
                u = work.tile([124, CH], dt.float32, tag="u")
                nc.vector.tensor_sub(u[:], kd2[:], kf[:])
                ec_a = 2 * t * GG + half * CH
                ec_b = (2 * t + 1) * GG + half * CH
                nc.scalar.activation(rhs_c[0:DIS, ec_a:ec_a + CH], u[0:DIS, :],
                                     AF.Sin, scale=float(2.0 * np.pi))
                nc.scalar.activation(rhs_c[0:DIS, ec_b:ec_b + CH], u[64:124, :],
                                     AF.Sin, scale=float(2.0 * np.pi))

        # first graph pair up front; remaining pairs interleave into layer 0
        emit_fd(0)

        if DEBUG:
            nc.sync.dma_start(dbg_fd[:, :], rhs_c[:])
            nc.sync.dma_start(dbg_nf0[:, :, :], nf_f[:])
        gf = work.tile([128, MT, GPC], dt.float32, tag="gf", bufs=1)
        # ---- message-passing layers
        z1_tiles = {}

        def emit_wc(i, c):
            ecol = c * CQ
            z1_ps = ps.tile([128, MT, CQ], dt.float32, tag="z1",
                            name=f"z1ps_{i}_{c}")
            for m in range(MT):
                nc.tensor.matmul(z1_ps[:, m, :],
                                 wb[0:DIS, o[f"wc{i}"] + m * 128: o[f"wc{i}"] + m * 128 + 128],
                                 rhs_c[:, ecol:ecol + CQ], start=True, stop=False)
            z1_tiles[(i, c)] = z1_ps

        for i in range(L):
            # Hi' = W1hi^T nf + (lat_ip, 1) @ wcl per graph (lat+b1 folded in).
            # NOTE: accumulation groups sharing a PSUM bank must not interleave
            # (start=True clears the whole bank's has_written bits), so each
            # m's group is emitted contiguously.
            hi_ps = ps.tile([128, MT, NPC], dt.float32, tag="z1")
            for m in range(MT):
                for kt in range(MT):
                    nc.tensor.matmul(hi_ps[:, m, :], wB(f"w1hi{i}", kt, m),
                                     nf_b[:, kt, :], start=(kt == 0), stop=False)
                nc.tensor.matmul(hi_ps[:, m, :],
                                 wb[0:10, o[f"wcl{i}"] + m * 128: o[f"wcl{i}"] + m * 128 + 128],
                                 latip1[:, :, None].broadcast_to([10, GPC, A]),
                                 start=False, stop=True)
            hi_b = work.tile([128, MT, NPC], dt.bfloat16, tag="hib")
            nc.scalar.copy(hi_b[:, :, 0:A], hi_ps[:, :, 0:A])
            nc.scalar.copy(hi_b[:, :, A:], hi_ps[:, :, A:])
            if DEBUG and i == 0:
                nc.sync.dma_start(dbg_hi[:, :, :], hi_b[:])
            hj_ps = ps.tile([128, MT, NPC], dt.float32, tag="ef")
            for m in range(MT):
                for kt in range(MT):
                    nc.tensor.matmul(hj_ps[:, m, :], wB(f"w1hj{i}", kt, m),
                                     nf_b[:, kt, :], start=(kt == 0), stop=(kt == 3))
            hj_b = work.tile([128, MT, NPC], dt.bfloat16, tag="hjb")
            nc.vector.tensor_copy(hj_b[:, :, 0:A], hj_ps[:, :, 0:A])
            nc.vector.tensor_copy(hj_b[:, :, A:], hj_ps[:, :, A:])
            if DEBUG and i == 0:
                nc.sync.dma_start(dbg_hj[:, :, :], hj_b[:])

            agg = work.tile([128, MT, NPC], dt.bfloat16, tag="agg")
            # software-pipelined chunk loop: z1(c+1) emitted before mm2(c) so
            # the PE has work while silu1(c) runs. Accumulation groups that
            # share a PSUM bank (m-pairs at CQ=256) are emitted contiguously:
            # per m, Wc then ident-broadcast, closed before the next m starts.
            z1_tiles = {}

            def emit_z1(c):
                g, q = c // 4, c % 4
                i0 = g * A + 8 * q
                ecol = c * CQ
                hs = work.tile([128, MT, CQ], dt.bfloat16, tag="hs", bufs=3,
                               name=f"hs_{i}_{c}")
                nc.vector.tensor_add(
                    hs[:].rearrange("p mt (i j) -> p mt i j", j=A),
                    hi_b[:, :, i0:i0 + 8][:, :, :, None].broadcast_to([128, MT, 8, A]),
                    hj_b[:, :, g * A:(g + 1) * A][:, :, None, :].broadcast_to([128, MT, 8, A]))
                z1_ps = ps.tile([128, MT, CQ], dt.float32, tag="z1",
                                name=f"z1ps_{i}_{c}")
                for m in range(MT):
                    nc.tensor.matmul(z1_ps[:, m, :],
                                     wb[0:DIS, o[f"wc{i}"] + m * 128: o[f"wc{i}"] + m * 128 + 128],
                                     rhs_c[:, ecol:ecol + CQ], start=True, stop=False)
                    nc.tensor.matmul(z1_ps[:, m, :], ident, hs[:, m, :],
                                     start=False, stop=True)
                z1_tiles[c] = z1_ps

            emit_z1(0)
            for c in range(NCH):
                z1_ps = z1_tiles.pop(c)
                z1s = work.tile([128, MT, CQ], dt.bfloat16, tag="z1s", bufs=2)
                nc.scalar.activation(z1s[:, 0:2, :], z1_ps[:, 0:2, :], AF.Silu)
                nc.scalar.activation(z1s[:, 2:4, :], z1_ps[:, 2:4, :], AF.Silu)
                if DEBUG and i == 0 and c == 0:
                    nc.sync.dma_start(dbg_z1s[:, :, :], z1s[:])
                if i == 0 and c in (2, 10, 18):
                    emit_fd((c + 6) // 8)
                if c + 1 < NCH:
                    emit_z1(c + 1)
                ef_ps = ps.tile([128, MT, CQ], dt.float32, tag="ef")
                for m in range(MT):
                    for kt in range(MT):
                        nc.tensor.matmul(ef_ps[:, m, :], wB(f"w2{i}", kt, m),
                                         z1s[:, kt, :], start=(kt == 0), stop=(kt == 3))
                efs = work.tile([128, MT, CQ], dt.bfloat16, tag="efs", bufs=2)
                if zero_bias:
                    nc.scalar.activation(efs[:], ef_ps[:], AF.Silu)
                else:
                    for m in range(MT):
                        nc.scalar.activation(efs[:, m, :], ef_ps[:, m, :], AF.Silu,
                                             bias=cB(f"b2_{i}", m))
                with nc.allow_low_precision("bf16 agg output; DVE accumulates fp32"):
                    nc.vector.tensor_reduce(
                        agg[:, :, c * 8:(c + 1) * 8],
                        efs[:].rearrange("p mt (n j) -> p mt n j", j=A),
                        axis=AX.X, op=OP.add)

            if DEBUG and i == 0:
                nc.sync.dma_start(dbg_agg[:, :, :], agg[:])
            n1_ps = ps.tile([128, MT, NPC], dt.float32, tag="z1")
            for m in range(MT):
                for kt in range(8):
                    rhs = nf_b[:, kt, :] if kt < 4 else agg[:, kt - 4, :]
                    nc.tensor.matmul(n1_ps[:, m, :], wB(f"nw1{i}", kt, m), rhs,
                                     start=(kt == 0), stop=(kt == 7))
            n1s = work.tile([128, MT, NPC], dt.bfloat16, tag="z1s", bufs=2)
            if zero_bias:
                nc.scalar.activation(n1s[:], n1_ps[:], AF.Silu)
            else:
                for m in range(MT):
                    nc.scalar.activation(n1s[:, m, :], n1_ps[:, m, :], AF.Silu,
                                         bias=cB(f"nb1_{i}", m))
            n2_ps = ps.tile([128, MT, NPC], dt.float32, tag="ef")
            for m in range(MT):
                for kt in range(MT):
                    nc.tensor.matmul(n2_ps[:, m, :], wB(f"nw2{i}", kt, m),
                                     n1s[:, kt, :], start=(kt == 0), stop=(kt == 3))
            res = work.tile([128, MT, NPC], dt.float32, tag="resf")
            for m in range(MT):
                if zero_bias:
                    nc.scalar.activation(res[:, m, :], n2_ps[:, m, :], AF.Silu)
                else:
                    nc.scalar.activation(res[:, m, :], n2_ps[:, m, :], AF.Silu,
                                         bias=cB(f"nb2_{i}", m))
                nc.vector.tensor_add(nf_f[:, m, :], nf_f[:, m, :], res[:, m, :])
                nc.vector.tensor_copy(nf_b[:, m, :], nf_f[:, m, :])
                if i == L - 1:
                    with nc.allow_low_precision("graph mean"):
                        nc.vector.tensor_reduce(
                            gf[:, m, :],
                            nf_f[:, m, :].rearrange("p (g j) -> p g j", j=A),
                            axis=AX.X, op=OP.add)
            if DEBUG and i == 0:
                nc.sync.dma_start(dbg_nf[:, :, :], nf_f[:])

        # ---- outputs
        co_ps = ps.tile([3, NPC], dt.float32, tag="z1")
        for kt in range(MT):
            nc.tensor.matmul(co_ps[:], wb[:, o["coordw"] + kt * 3: o["coordw"] + kt * 3 + 3],
                             nf_b[:, kt, :], start=(kt == 0), stop=(kt == 3))
        co_sb = work.tile([3, NPC], dt.float32, tag="cosb", bufs=1)
        nc.vector.tensor_copy(co_sb[:], co_ps[:])
        nc.sync.dma_start(co_d[:, :], co_sb[:])

        gfb = work.tile([128, MT, GPC], dt.bfloat16, tag="gfb", bufs=1)
        nc.vector.tensor_copy(gfb[:], gf[:])
        lp_ps = ps.tile([9, GPC], dt.float32, tag="ef")
        for kt in range(MT):
            nc.tensor.matmul(lp_ps[:], wb[:, o["latw"] + kt * 9: o["latw"] + kt * 9 + 9],
                             gfb[:, kt, :], start=(kt == 0), stop=(kt == 3))
        lp_sb = work.tile([9, GPC], dt.float32, tag="lpsb", bufs=1)
        nc.vector.tensor_copy(lp_sb[:], lp_ps[:])
        mt_ps = ps.tile([GPC, 9], dt.float32, tag="z1")
        nc.tensor.matmul(mt_ps[:], lp_sb[:], cb[0:9, co["id9"]:co["id9"] + 9],
                         start=True, stop=True)
        m_sb = work.tile([GPC, 9], dt.float32, tag="msb", bufs=1)
        nc.vector.tensor_copy(m_sb[:], mt_ps[:])
        # lattice_out[g, i*3+k] = sum_j M[g, i*3+j] * lt[g, j*3+k]
        lo_sb = work.tile([GPC, 9], dt.float32, tag="losb", bufs=1)
        lo_t = work.tile([GPC, 9], dt.float32, tag="lotb", bufs=1)
        lo3 = lo_sb[:].rearrange("g (i k) -> g i k", i=3)
        lt3 = lo_t[:].rearrange("g (i k) -> g i k", i=3)
        m3d = m_sb[:].rearrange("g (i j) -> g i j", i=3)

        def m_view(j):
            return m3d[:, :, j].unsqueeze(2).broadcast_to([GPC, 3, 3])

        def l_view(j):
            return lt3d[:, j, :].unsqueeze(1).broadcast_to([GPC, 3, 3])

        nc.vector.tensor_mul(lo3, m_view(0), l_view(0))
        nc.vector.tensor_mul(lt3, m_view(1), l_view(1))
        nc.vector.tensor_add(lo3, lo_sb[:].rearrange("g (i k) -> g i k", i=3), lt3)
        nc.vector.tensor_mul(lt3, m_view(2), l_view(2))
        nc.vector.tensor_add(lo3, lo_sb[:].rearrange("g (i k) -> g i k", i=3), lt3)
        nc.sync.dma_start(lo_d[:, :], lo_sb[:])

    _legalize_waits(nc)
    return nc


_CACHED = {}


def _get_program():
    if "nc" not in _CACHED:
        _CACHED["nc"] = _build_program(zero_bias=_CACHED.get("zero_bias", False))
    return _CACHED["nc"]


def _get_runner():
    """Cached jitted shard_map executable over the 8 NeuronCores.

    Adapted from concourse.bass2jax.run_bass_via_pjrt so repeat calls reuse
    the compiled NEFF (that path re-jits per call)."""
    if "runner" in _CACHED:
        return _CACHED["runner"]
    import jax
    import concourse.mybir as mybir
    from concourse import bass2jax
    from jax.sharding import Mesh, PartitionSpec
    from jax.experimental.shard_map import shard_map

    nc = _get_program()
    bass2jax.install_neuronx_cc_hook()
    partition_name = nc.partition_id_tensor.name if nc.partition_id_tensor else None
    in_names, out_names, out_avals, zero_outs = [], [], [], []
    for alloc in nc.m.functions[0].allocations:
        if not isinstance(alloc, mybir.MemoryLocationSet):
            continue
        name = alloc.memorylocations[0].name
        if alloc.kind == "ExternalInput":
            if name != partition_name:
                in_names.append(name)
        elif alloc.kind == "ExternalOutput":
            out_names.append(name)
            shape = tuple(alloc.tensor_shape)
            dtype = mybir.dt.np(alloc.dtype)
            out_avals.append(jax.core.ShapedArray(shape, dtype))
            zero_outs.append(np.zeros(shape, dtype))
    n_params, n_outs = len(in_names), len(out_avals)
    all_in_names = list(in_names) + list(out_names)
    if partition_name is not None:
        all_in_names.append(partition_name)

    def _body(*args):
        operands = list(args)
        if partition_name is not None:
            operands.append(bass2jax.partition_id_tensor())
        outs = bass2jax._bass_exec_p.bind(
            *operands,
            out_avals=tuple(out_avals),
            in_names=tuple(all_in_names),
            out_names=tuple(out_names),
            lowering_input_output_aliases=(),
            sim_require_finite=True,
            sim_require_nnan=True,
            nc=nc,
        )
        return tuple(outs)

    devices = jax.devices()[:NCORES]
    mesh = Mesh(np.asarray(devices), ("core",))
    in_specs = (PartitionSpec("core"),) * (n_params + n_outs)
    out_specs = (PartitionSpec("core"),) * len(out_names)
    donate = tuple(range(n_params, n_params + n_outs))
    sharded = jax.jit(
        shard_map(_body, mesh=mesh, in_specs=in_specs, out_specs=out_specs,
                  check_rep=False),
        donate_argnums=donate, keep_unused=True)

    runner = dict(fn=sharded, mesh=mesh, in_names=in_names, out_names=out_names,
                  out_avals=out_avals, zero_outs=zero_outs)
    _CACHED["runner"] = runner
    return runner


def _make_in_maps(inputs):
    wb, cb = _host_blobs(inputs)
    frac = np.asarray(inputs["frac_coords"], np.float32)      # [N, 3]
    t = np.asarray(inputs["t"], np.float32)                   # [B, LAT]
    latt = np.asarray(inputs["lattices"], np.float32)         # [B, 3, 3]
    at = np.asarray(inputs["atom_types"])                     # [N]
    in_maps = []
    for c in range(NCORES):
        nsl = slice(c * NPC, (c + 1) * NPC)
        gsl = slice(c * GPC, (c + 1) * GPC)
        onehot = np.zeros((100, NPC), np.float32)
        onehot[at[nsl] - 1, np.arange(NPC)] = 1.0
        in_maps.append(dict(
            wb=wb, cb=cb,
            fr=np.ascontiguousarray(frac[nsl].T),
            oh=onehot.astype(_bf16),
            tt=np.ascontiguousarray(t[gsl].T).astype(_bf16),
            lt=np.ascontiguousarray(latt[gsl].reshape(GPC, 9)),
        ))
    return in_maps


def _run(in_maps, device_inputs=None):
    import jax
    r = _get_runner()
    n_cores = NCORES
    if device_inputs is None:
        concat_in = [
            np.concatenate([np.asarray(in_maps[c][name]) for c in range(n_cores)], axis=0)
            for name in r["in_names"]
        ]
    else:
        concat_in = device_inputs
    concat_zeros = [
        np.zeros((n_cores * z.shape[0], *z.shape[1:]), z.dtype) for z in r["zero_outs"]
    ]
    out_arrs = r["fn"](*concat_in, *concat_zeros)
    jax.block_until_ready(out_arrs)
    return [
        {name: np.asarray(out_arrs[i]).reshape(n_cores, *r["out_avals"][i].shape)[c]
         for i, name in enumerate(r["out_names"])}
        for c in range(n_cores)
    ]


def _assemble(results):
    coord_out = np.empty((N, 3), np.float32)
    lat_out = np.empty((B, 3, 3), np.float32)
    for c in range(NCORES):
        out = results[c]
        coord_out[c * NPC:(c + 1) * NPC] = out["co"].T
        lat_out[c * GPC:(c + 1) * GPC] = out["lo"].reshape(GPC, 3, 3)
    return (lat_out, coord_out)


def _set_zero_bias(inputs):
    if "nc" in _CACHED:
        return
    zb = all(
        not np.any(np.asarray(inputs[k]))
        for k in ("b_latent", "edge_b2", "node_b1", "node_b2"))
    _CACHED["zero_bias"] = zb


def kernel(**inputs):
    if not _structure_ok(inputs):
        return _fallback(inputs)
    _set_zero_bias(inputs)
    results = _run(_make_in_maps(inputs))
    return _assemble(results)


def bench_chain(inputs, k=32, reps=5):
    """Per-execution device time via K chained NEFF executions in one jit.

    Output buffers of call i feed call i+1's output-init operands, creating a
    dependency chain (no CSE, serialized execution). Per-iter time =
    (t_chain(k) - t_chain(1)) / (k - 1)."""
    import time
    import jax
    import concourse.mybir as mybir
    from concourse import bass2jax
    from jax.sharding import Mesh, PartitionSpec, NamedSharding
    from jax.experimental.shard_map import shard_map

    nc = _get_program()
    bass2jax.install_neuronx_cc_hook()
    partition_name = nc.partition_id_tensor.name if nc.partition_id_tensor else None
    in_names, out_names, out_avals, zero_outs = [], [], [], []
    for alloc in nc.m.functions[0].allocations:
        if not isinstance(alloc, mybir.MemoryLocationSet):
            continue
        name = alloc.memorylocations[0].name
        if alloc.kind == "ExternalInput":
            if name != partition_name:
                in_names.append(name)
        elif alloc.kind == "ExternalOutput":
            out_names.append(name)
            shape = tuple(alloc.tensor_shape)
            dtype = mybir.dt.np(alloc.dtype)
            out_avals.append(jax.core.ShapedArray(shape, dtype))
            zero_outs.append(np.zeros(shape, dtype))
    all_in_names = list(in_names) + list(out_names)
    if partition_name is not None:
        all_in_names.append(partition_name)

    def _call(params, outs_init):
        operands = list(params) + list(outs_init)
        if partition_name is not None:
            operands.append(bass2jax.partition_id_tensor())
        return bass2jax._bass_exec_p.bind(
            *operands, out_avals=tuple(out_avals), in_names=tuple(all_in_names),
            out_names=tuple(out_names), lowering_input_output_aliases=(),
            sim_require_finite=True, sim_require_nnan=True, nc=nc)

    def _chain_body(kk):
        def body(*args):
            n_p = len(in_names)
            params, outs = args[:n_p], list(args[n_p:])
            for _ in range(kk):
                outs = list(_call(params, outs))
            return tuple(outs)
        return body

    devices = jax.devices()[:NCORES]
    mesh = Mesh(np.asarray(devices), ("core",))
    sharding = NamedSharding(mesh, PartitionSpec("core"))
    in_maps = _make_in_maps(inputs)
    dev_in = [jax.device_put(
        np.concatenate([np.asarray(in_maps[c][name]) for c in range(NCORES)], axis=0),
        sharding) for name in in_names]
    dev_zero = [jax.device_put(
        np.zeros((NCORES * z.shape[0], *z.shape[1:]), z.dtype), sharding)
        for z in zero_outs]
    n_all = len(dev_in) + len(dev_zero)

    def timed(kk):
        f = jax.jit(shard_map(_chain_body(kk), mesh=mesh,
                              in_specs=(PartitionSpec("core"),) * n_all,
                              out_specs=(PartitionSpec("core"),) * len(out_names),
                              check_rep=False), keep_unused=True)
        r = f(*dev_in, *dev_zero); jax.block_until_ready(r)  # compile+warm
        ts = []
        for _ in range(reps):
            t0 = time.perf_counter()
            r = f(*dev_in, *dev_zero)
            jax.block_until_ready(r)
            ts.append(time.perf_counter() - t0)
        return min(ts)

    t1 = timed(1)
    tk = timed(k)
    per_iter_ns = (tk - t1) / (k - 1) * 1e9
    return per_iter_ns, t1 * 1e9, tk * 1e9


def bench(inputs, iters=30):
    """Time repeated device executions with device-resident inputs.

    Returns (per_iter_ns_median, per_iter_ns_min, all_times)."""
    import time
    import jax
    from jax.sharding import NamedSharding, PartitionSpec
    r = _get_runner()
    in_maps = _make_in_maps(inputs)
    sharding = NamedSharding(r["mesh"], PartitionSpec("core"))
    dev_in = [
        jax.device_put(
            np.concatenate([np.asarray(in_maps[c][name]) for c in range(NCORES)], axis=0),
            sharding)
        for name in r["in_names"]
    ]
    _run(in_maps, device_inputs=dev_in)  # warmup / compile
    times = []
    for _ in range(iters):
        t0 = time.perf_counter()
        _run(in_maps, device_inputs=dev_in)
        times.append((time.perf_counter() - t0) * 1e9)
    times.sort()
    return times[len(times) // 2], times[0], times
